# revision 22
# baseline (speedup 1.0000x reference)
"""Trainium2 Bass kernel for nn_ChannelMixingConv1D.

Reference computation (B=64, C_in=128, C_out=256, L=2048, fp32):
    y = depthwise_conv1d(x, dw_w, k=3, pad=SAME) + dw_b          # [B, C_in, L]
    z = mix_w @ y + mix_b                                        # [B, C_out, L]
    out = relu(batchnorm(z) * gamma + beta)    # BN over (batch, length), biased var

Kernel strategy (8 NeuronCores, data-parallel over batch, 8 batches/core):
  * Fold the depthwise conv into the 1x1 mix:
        z[b,o,l] = sum_k sum_c (mix_w[o,c] * dw_w[c,k]) * x[b,c,l+k-1]
    i.e. 3 shifted matmuls accumulating in PSUM with host-prefolded weights.
  * The conv biases (dw_b, mix_b) shift per-channel means only, which BN
    subtracts exactly -> they drop out and are never computed.
  * Matmuls run in bf16 (x and the folded weights are converted on host):
    full PE rate + fast weight load; f32r measured ~2x slower per matmul.
  * Per (batch, out-chunk) tile: 12 matmuls -> PSUM [128, 2048];
    DVE evacuates PSUM->SBUF with a sum(z) accumulator while ACT computes
    Square with a sum(z^2) accumulator, both reading PSUM in parallel.
  * BN batch stats: tiny [128, 4] AllGather across the 8 cores (cheaper than
    AllReduce) + local reduction; a dummy warm-up collective runs under
    phase 1 to hide the ncfw wake-up latency.
  * rsqrt = ACT Sqrt + DVE reciprocal + one Newton step (ACT sqrt has a loose
    ULP budget).
  * Final out = relu(z*a + b): fused ACT activation for half the tiles, DVE
    two-op path for the other half, overlapped with the output DMA.
"""

import numpy as np

B, C_IN, C_OUT, L = 64, 128, 256, 2048
N_CORES = 8
B_PER = B // N_CORES  # 8 batches per core
EPS = 1e-5
N_TOT = float(B * L)  # BN sample count per channel
P = 128
LPAD = L + 2  # one zero column of padding each side
N_LC = L // 512  # 4 free-dim chunks of 512

_CACHE = {}


def _build_nc():
    import concourse.bacc as bacc
    import concourse.tile as tile
    from concourse import mybir

    f32 = mybir.dt.float32
    bf16 = mybir.dt.bfloat16
    AF = mybir.ActivationFunctionType
    ALU = mybir.AluOpType

    nc = bacc.Bacc("TRN2", debug=False, num_devices=N_CORES)

    # x arrives host-padded with one zero column each side, pre-cast to bf16.
    x_d = nc.dram_tensor("x", [B_PER, C_IN, LPAD], bf16, kind="ExternalInput")
    # Pre-folded lhsT weights: wt[:, (oc*3+k)*128 : +128] = (mix_w * dw_w[:,k]).T chunk
    wt_d = nc.dram_tensor("wt", [C_IN, 6 * P], bf16, kind="ExternalInput")
    # gamma/beta split by out-chunk: cols = [g0, g1, b0, b1]
    gb_d = nc.dram_tensor("gb", [P, 4], f32, kind="ExternalInput")
    out_d = nc.dram_tensor("out", [B_PER, C_OUT, L], f32, kind="ExternalOutput")

    x_ap = x_d.ap()
    out_ap = out_d.ap()

    with tile.TileContext(nc) as tc:
        with (
            tc.tile_pool(name="consts", bufs=1) as consts,
            tc.tile_pool(name="xin", bufs=8) as xin,
            tc.tile_pool(name="zbuf", bufs=1) as zbuf,
            tc.tile_pool(name="scr", bufs=2) as scrpool,
            tc.tile_pool(name="stats", bufs=1) as stats,
            tc.tile_pool(name="psum", bufs=2, space="PSUM") as pspool,
            tc.tile_pool(name="dram", bufs=1, space="DRAM") as dram,
        ):
            # ---- constants first (tiny; the first matmul needs wt) ----
            wt_sb = consts.tile([P, 6 * P], bf16)
            nc.sync.dma_start(out=wt_sb, in_=wt_d.ap())
            gb_sb = consts.tile([P, 4], f32)
            nc.sync.dma_start(out=gb_sb, in_=gb_d.ap())

            # ---- prefetch all 8 x batches (each gets its own slot) ----
            x_tiles = []
            for b in range(B_PER):
                xt = xin.tile([P, LPAD], bf16, tag="xt", name=f"xt{b}")
                nc.sync.dma_start(out=xt, in_=x_ap[b])
                x_tiles.append(xt)

            # ---- warm-up collective: wakes ncfw while phase 1 runs ----
            warm_in = dram.tile([P, 1], f32)
            warm_out = dram.tile([P * N_CORES, 1], f32)
            nc.sync.dma_start(out=warm_in, in_=gb_d.ap()[:, 0:1])
            nc.gpsimd.collective_compute(
                "AllGather",
                ALU.bypass,
                replica_groups=[list(range(N_CORES))],
                ins=[warm_in.opt()],
                outs=[warm_out.opt()],
            )

            # per-(oc) accumulator columns for sum(z) / sum(z^2), one per batch
            zsum = [stats.tile([P, B_PER], f32, name=f"zsum{oc}") for oc in range(2)]
            qsum = [stats.tile([P, B_PER], f32, name=f"qsum{oc}") for oc in range(2)]

            # ---- phase 1: matmuls + evacuation + stats ----
            z_tiles = {}
            for b in range(B_PER):
                xt = x_tiles[b]
                for oc in range(2):
                    pt = pspool.tile([P, L], f32, tag="pt")
                    for lc in range(N_LC):
                        for k in range(3):
                            nc.tensor.matmul(
                                out=pt[:, lc * 512 : (lc + 1) * 512],
                                lhsT=wt_sb[
                                    :, (oc * 3 + k) * P : (oc * 3 + k + 1) * P
                                ],
                                rhs=xt[:, lc * 512 + k : lc * 512 + k + 512],
                                start=(k == 0),
                                stop=(k == 2),
                            )
                    zt = zbuf.tile([P, L], f32, tag=f"z{b}_{oc}", name=f"z{b}_{oc}")
                    z_tiles[(b, oc)] = zt
                    # DVE: copy PSUM->SBUF, accumulate sum(z)
                    nc.vector.tensor_scalar(
                        out=zt,
                        in0=pt,
                        scalar1=0.0,
                        scalar2=None,
                        op0=ALU.add,
                        op1=ALU.add,  # reduce op for accum_out
                        accum_out=zsum[oc][:, b : b + 1],
                    )
                    # ACT: z^2 into scratch, accumulate sum(z^2). Reads the
                    # SBUF copy (not PSUM) so the PSUM slot is released by
                    # the DVE evacuation alone -- keeps the PE from stalling
                    # on ACT at batch boundaries.
                    scr = scrpool.tile([P, L], f32, tag="scr")
                    nc.scalar.activation(
                        out=scr,
                        in_=zt,
                        func=AF.Square,
                        accum_out=qsum[oc][:, b : b + 1],
                    )

            # ---- phase 2: combine per-batch sums, all-gather, BN constants ----
            part = stats.tile([P, 4], f32)
            nc.vector.tensor_reduce(
                out=part[:, 0:1], in_=zsum[0], axis=mybir.AxisListType.X, op=ALU.add
            )
            nc.vector.tensor_reduce(
                out=part[:, 1:2], in_=zsum[1], axis=mybir.AxisListType.X, op=ALU.add
            )
            nc.vector.tensor_reduce(
                out=part[:, 2:3], in_=qsum[0], axis=mybir.AxisListType.X, op=ALU.add
            )
            nc.vector.tensor_reduce(
                out=part[:, 3:4], in_=qsum[1], axis=mybir.AxisListType.X, op=ALU.add
            )

            cc_in = dram.tile([P, 4], f32)
            cc_out = dram.tile([P * N_CORES, 4], f32)
            # SWDGE so the bounce write and the collective trigger share the
            # gpsimd queue (no extra cross-engine hop on the critical path)
            nc.gpsimd.dma_start(out=cc_in, in_=part)
            nc.gpsimd.collective_compute(
                "AllGather",
                ALU.bypass,
                replica_groups=[list(range(N_CORES))],
                ins=[cc_in.opt()],
                outs=[cc_out.opt()],
            )
            # gathered layout row-major [(r p), c] -> SBUF [p, r, c]
            allp = stats.tile([P, N_CORES, 4], f32)
            nc.sync.dma_start(
                out=allp, in_=cc_out.rearrange("(r p) c -> p r c", p=P)
            )
            tot = stats.tile([P, 4], f32)
            # reduce over cores: view as [p, c, r] and reduce innermost
            nc.vector.tensor_reduce(
                out=tot,
                in_=allp.transpose([0, 2, 1]),
                axis=mybir.AxisListType.X,
                op=ALU.add,
            )

            # mean, E[z^2] -> var -> rsqrt (Newton-refined) -> a, b
            mean = stats.tile([P, 2], f32)
            nc.vector.tensor_scalar(
                out=mean, in0=tot[:, 0:2], scalar1=1.0 / N_TOT, scalar2=None,
                op0=ALU.mult,
            )
            vpe = stats.tile([P, 2], f32)
            nc.vector.tensor_scalar(
                out=vpe, in0=tot[:, 2:4], scalar1=1.0 / N_TOT, scalar2=EPS,
                op0=ALU.mult, op1=ALU.add,
            )
            msq = stats.tile([P, 2], f32)
            nc.vector.tensor_tensor(out=msq, in0=mean, in1=mean, op=ALU.mult)
            nc.vector.tensor_tensor(out=vpe, in0=vpe, in1=msq, op=ALU.subtract)

            std = stats.tile([P, 2], f32)
            nc.scalar.activation(out=std, in_=vpe, func=AF.Sqrt)
            r0 = stats.tile([P, 2], f32)
            nc.vector.reciprocal(out=r0, in_=std)
            # one Newton step for rsqrt: r = r0 * (1.5 - 0.5 * v * r0^2)
            t = stats.tile([P, 2], f32)
            nc.vector.tensor_tensor(out=t, in0=vpe, in1=r0, op=ALU.mult)
            nc.vector.tensor_tensor(out=t, in0=t, in1=r0, op=ALU.mult)
            nc.vector.tensor_scalar(
                out=t, in0=t, scalar1=-0.5, scalar2=1.5, op0=ALU.mult, op1=ALU.add
            )
            rr = stats.tile([P, 2], f32)
            nc.vector.tensor_tensor(out=rr, in0=r0, in1=t, op=ALU.mult)

            a_t = stats.tile([P, 2], f32)
            nc.vector.tensor_tensor(out=a_t, in0=gb_sb[:, 0:2], in1=rr, op=ALU.mult)
            b_t = stats.tile([P, 2], f32)
            nc.vector.tensor_tensor(out=b_t, in0=mean, in1=a_t, op=ALU.mult)
            nc.vector.tensor_tensor(
                out=b_t, in0=gb_sb[:, 2:4], in1=b_t, op=ALU.subtract
            )

            # ---- phase 3: normalize + relu + store (split ACT / DVE) ----
            for b in range(B_PER):
                for oc in range(2):
                    zt = z_tiles[(b, oc)]
                    if (b + oc) % 2 == 0:
                        nc.scalar.activation(
                            out=zt,
                            in_=zt,
                            func=AF.Relu,
                            scale=a_t[:, oc : oc + 1],
                            bias=b_t[:, oc : oc + 1],
                        )
                    else:
                        nc.vector.tensor_scalar(
                            out=zt,
                            in0=zt,
                            scalar1=a_t[:, oc : oc + 1],
                            scalar2=b_t[:, oc : oc + 1],
                            op0=ALU.mult,
                            op1=ALU.add,
                        )
                        nc.vector.tensor_scalar(
                            out=zt, in0=zt, scalar1=0.0, scalar2=None, op0=ALU.max
                        )
                    nc.sync.dma_start(
                        out=out_ap[b, oc * P : (oc + 1) * P, :], in_=zt
                    )

    nc.compile()
    return nc


def _prepare_aux(dw_w, mix_w, gamma, beta):
    import ml_dtypes

    # lhsT chunk for (oc, k): (mix_w[oc*128:(oc+1)*128] * dw_w[:,0,k]).T -> [C_in, 128]
    dw = np.asarray(dw_w, dtype=np.float32)  # [C_in, 1, 3]
    mw = np.asarray(mix_w, dtype=np.float32)  # [C_out, C_in]
    chunks = []
    for oc in range(2):
        for k in range(3):
            wk = mw[oc * P : (oc + 1) * P, :] * dw[None, :, 0, k]  # [128, C_in]
            chunks.append(np.ascontiguousarray(wk.T))  # [C_in, 128]
    wt = np.concatenate(chunks, axis=1).astype(ml_dtypes.bfloat16)  # [C_in, 768]
    g = np.asarray(gamma, dtype=np.float32)
    bt = np.asarray(beta, dtype=np.float32)
    gb = np.stack([g[:P], g[P:], bt[:P], bt[P:]], axis=1).astype(np.float32)
    return np.ascontiguousarray(wt), np.ascontiguousarray(gb)


def kernel(x, dw_w, dw_b, mix_w, mix_b, gamma, beta):
    import ml_dtypes

    from concourse import bass_utils

    x = np.asarray(x, dtype=np.float32)
    x_pad = np.zeros((B, C_IN, LPAD), dtype=ml_dtypes.bfloat16)
    x_pad[:, :, 1 : 1 + L] = x.astype(ml_dtypes.bfloat16)
    wt, gb = _prepare_aux(dw_w, mix_w, gamma, beta)

    if "nc" not in _CACHE:
        _CACHE["nc"] = _build_nc()
    nc = _CACHE["nc"]

    in_maps = [
        {
            "x": np.ascontiguousarray(x_pad[r * B_PER : (r + 1) * B_PER]),
            "wt": wt,
            "gb": gb,
        }
        for r in range(N_CORES)
    ]
    import os

    extra = {}
    if os.environ.get("BASS_TRACE_ALL") == "1":
        extra = {"trace_cores": list(range(N_CORES)), "stitch_traces": True}
    res = bass_utils.run_bass_kernel_spmd(
        nc, in_maps, core_ids=list(range(N_CORES)), **extra
    )
    _CACHE["last_results"] = res
    out = np.concatenate([res.results[r]["out"] for r in range(N_CORES)], axis=0)
    return out


# revision 24
# speedup vs baseline: 1.0158x; 1.0158x over previous
"""Trainium2 Bass kernel for nn_ChannelMixingConv1D.

Reference computation (B=64, C_in=128, C_out=256, L=2048, fp32):
    y = depthwise_conv1d(x, dw_w, k=3, pad=SAME) + dw_b          # [B, C_in, L]
    z = mix_w @ y + mix_b                                        # [B, C_out, L]
    out = relu(batchnorm(z) * gamma + beta)    # BN over (batch, length), biased var

Kernel strategy (8 NeuronCores, data-parallel over batch, 8 batches/core):
  * Fold the depthwise conv into the 1x1 mix:
        z[b,o,l] = sum_k sum_c (mix_w[o,c] * dw_w[c,k]) * x[b,c,l+k-1]
    i.e. 3 shifted matmuls accumulating in PSUM with host-prefolded weights.
  * The conv biases (dw_b, mix_b) shift per-channel means only, which BN
    subtracts exactly -> they drop out and are never computed.
  * Matmuls run in bf16 (x and the folded weights are converted on host):
    full PE rate + fast weight load; f32r measured ~2x slower per matmul.
  * Per (batch, out-chunk) tile: 12 matmuls -> PSUM [128, 2048];
    DVE evacuates PSUM->SBUF with a sum(z) accumulator while ACT computes
    Square with a sum(z^2) accumulator, both reading PSUM in parallel.
  * BN batch stats: tiny [128, 4] AllGather across the 8 cores (cheaper than
    AllReduce) + local reduction; a dummy warm-up collective runs under
    phase 1 to hide the ncfw wake-up latency.
  * rsqrt = ACT Sqrt + DVE reciprocal + one Newton step (ACT sqrt has a loose
    ULP budget).
  * Final out = relu(z*a + b): fused ACT activation for half the tiles, DVE
    two-op path for the other half, overlapped with the output DMA.
"""

import numpy as np

B, C_IN, C_OUT, L = 64, 128, 256, 2048
N_CORES = 8
B_PER = B // N_CORES  # 8 batches per core
EPS = 1e-5
N_TOT = float(B * L)  # BN sample count per channel
P = 128
LPAD = L + 2  # one zero column of padding each side
N_LC = L // 512  # 4 free-dim chunks of 512

_CACHE = {}


def _build_nc():
    import concourse.bacc as bacc
    import concourse.tile as tile
    from concourse import mybir

    f32 = mybir.dt.float32
    bf16 = mybir.dt.bfloat16
    AF = mybir.ActivationFunctionType
    ALU = mybir.AluOpType

    nc = bacc.Bacc("TRN2", debug=False, num_devices=N_CORES)

    # x arrives host-padded with one zero column each side, pre-cast to bf16.
    x_d = nc.dram_tensor("x", [B_PER, C_IN, LPAD], bf16, kind="ExternalInput")
    # Pre-folded lhsT weights: wt[:, (oc*3+k)*128 : +128] = (mix_w * dw_w[:,k]).T chunk
    wt_d = nc.dram_tensor("wt", [C_IN, 6 * P], bf16, kind="ExternalInput")
    # gamma/beta split by out-chunk: cols = [g0, g1, b0, b1]
    gb_d = nc.dram_tensor("gb", [P, 4], f32, kind="ExternalInput")
    out_d = nc.dram_tensor("out", [B_PER, C_OUT, L], f32, kind="ExternalOutput")

    x_ap = x_d.ap()
    out_ap = out_d.ap()

    with tile.TileContext(nc) as tc:
        with (
            tc.tile_pool(name="consts", bufs=1) as consts,
            tc.tile_pool(name="xin", bufs=8) as xin,
            tc.tile_pool(name="zbuf", bufs=1) as zbuf,
            tc.tile_pool(name="scr", bufs=2) as scrpool,
            tc.tile_pool(name="stats", bufs=1) as stats,
            tc.tile_pool(name="psum", bufs=2, space="PSUM") as pspool,
            tc.tile_pool(name="dram", bufs=1, space="DRAM") as dram,
        ):
            # ---- constants first (tiny; the first matmul needs wt) ----
            wt_sb = consts.tile([P, 6 * P], bf16)
            nc.sync.dma_start(out=wt_sb, in_=wt_d.ap())
            gb_sb = consts.tile([P, 4], f32)
            nc.sync.dma_start(out=gb_sb, in_=gb_d.ap())

            # ---- prefetch all 8 x batches (each gets its own slot) ----
            # alternate the two HWDGE rings so two loads stream in parallel
            x_tiles = []
            for b in range(B_PER):
                xt = xin.tile([P, LPAD], bf16, tag="xt", name=f"xt{b}")
                eng = nc.sync if b % 2 == 0 else nc.scalar
                eng.dma_start(out=xt, in_=x_ap[b])
                x_tiles.append(xt)

            # ---- warm-up collective: wakes ncfw while phase 1 runs ----
            warm_in = dram.tile([P, 1], f32)
            warm_out = dram.tile([P * N_CORES, 1], f32)
            nc.gpsimd.dma_start(out=warm_in, in_=gb_d.ap()[:, 0:1])
            nc.gpsimd.collective_compute(
                "AllGather",
                ALU.bypass,
                replica_groups=[list(range(N_CORES))],
                ins=[warm_in.opt()],
                outs=[warm_out.opt()],
            )

            # per-(oc) accumulator columns for sum(z) / sum(z^2), one per batch
            zsum = [stats.tile([P, B_PER], f32, name=f"zsum{oc}") for oc in range(2)]
            qsum = [stats.tile([P, B_PER], f32, name=f"qsum{oc}") for oc in range(2)]

            # ---- phase 1: matmuls + evacuation + stats ----
            z_tiles = {}
            for b in range(B_PER):
                xt = x_tiles[b]
                for oc in range(2):
                    pt = pspool.tile([P, L], f32, tag="pt")
                    for lc in range(N_LC):
                        for k in range(3):
                            nc.tensor.matmul(
                                out=pt[:, lc * 512 : (lc + 1) * 512],
                                lhsT=wt_sb[
                                    :, (oc * 3 + k) * P : (oc * 3 + k + 1) * P
                                ],
                                rhs=xt[:, lc * 512 + k : lc * 512 + k + 512],
                                start=(k == 0),
                                stop=(k == 2),
                            )
                    zt = zbuf.tile([P, L], f32, tag=f"z{b}_{oc}", name=f"z{b}_{oc}")
                    z_tiles[(b, oc)] = zt
                    # DVE: copy PSUM->SBUF, accumulate sum(z)
                    nc.vector.tensor_scalar(
                        out=zt,
                        in0=pt,
                        scalar1=0.0,
                        scalar2=None,
                        op0=ALU.add,
                        op1=ALU.add,  # reduce op for accum_out
                        accum_out=zsum[oc][:, b : b + 1],
                    )
                    # ACT: z^2 into scratch, accumulate sum(z^2). Reads the
                    # SBUF copy (not PSUM) so the PSUM slot is released by
                    # the DVE evacuation alone -- keeps the PE from stalling
                    # on ACT at batch boundaries.
                    scr = scrpool.tile([P, L], f32, tag="scr")
                    nc.scalar.activation(
                        out=scr,
                        in_=zt,
                        func=AF.Square,
                        accum_out=qsum[oc][:, b : b + 1],
                    )

            # ---- phase 2: combine per-batch sums, all-gather, BN constants ----
            part = stats.tile([P, 4], f32)
            nc.vector.tensor_reduce(
                out=part[:, 0:1], in_=zsum[0], axis=mybir.AxisListType.X, op=ALU.add
            )
            nc.vector.tensor_reduce(
                out=part[:, 1:2], in_=zsum[1], axis=mybir.AxisListType.X, op=ALU.add
            )
            nc.vector.tensor_reduce(
                out=part[:, 2:3], in_=qsum[0], axis=mybir.AxisListType.X, op=ALU.add
            )
            nc.vector.tensor_reduce(
                out=part[:, 3:4], in_=qsum[1], axis=mybir.AxisListType.X, op=ALU.add
            )

            cc_in = dram.tile([P, 4], f32)
            cc_out = dram.tile([P * N_CORES, 4], f32)
            # SWDGE so the bounce write and the collective trigger share the
            # gpsimd queue (no extra cross-engine hop on the critical path)
            nc.gpsimd.dma_start(out=cc_in, in_=part)
            nc.gpsimd.collective_compute(
                "AllGather",
                ALU.bypass,
                replica_groups=[list(range(N_CORES))],
                ins=[cc_in.opt()],
                outs=[cc_out.opt()],
            )
            # gathered layout row-major [(r p), c] -> SBUF [p, r, c]
            allp = stats.tile([P, N_CORES, 4], f32)
            nc.sync.dma_start(
                out=allp, in_=cc_out.rearrange("(r p) c -> p r c", p=P)
            )
            tot = stats.tile([P, 4], f32)
            # reduce over cores: view as [p, c, r] and reduce innermost
            nc.vector.tensor_reduce(
                out=tot,
                in_=allp.transpose([0, 2, 1]),
                axis=mybir.AxisListType.X,
                op=ALU.add,
            )

            # mean, E[z^2] -> var -> rsqrt (Newton-refined) -> a, b
            mean = stats.tile([P, 2], f32)
            nc.vector.tensor_scalar(
                out=mean, in0=tot[:, 0:2], scalar1=1.0 / N_TOT, scalar2=None,
                op0=ALU.mult,
            )
            vpe = stats.tile([P, 2], f32)
            nc.vector.tensor_scalar(
                out=vpe, in0=tot[:, 2:4], scalar1=1.0 / N_TOT, scalar2=EPS,
                op0=ALU.mult, op1=ALU.add,
            )
            msq = stats.tile([P, 2], f32)
            nc.vector.tensor_tensor(out=msq, in0=mean, in1=mean, op=ALU.mult)
            nc.vector.tensor_tensor(out=vpe, in0=vpe, in1=msq, op=ALU.subtract)

            std = stats.tile([P, 2], f32)
            nc.scalar.activation(out=std, in_=vpe, func=AF.Sqrt)
            r0 = stats.tile([P, 2], f32)
            nc.vector.reciprocal(out=r0, in_=std)
            # one Newton step for rsqrt: r = r0 * (1.5 - 0.5 * v * r0^2)
            t = stats.tile([P, 2], f32)
            nc.vector.tensor_tensor(out=t, in0=vpe, in1=r0, op=ALU.mult)
            nc.vector.tensor_tensor(out=t, in0=t, in1=r0, op=ALU.mult)
            nc.vector.tensor_scalar(
                out=t, in0=t, scalar1=-0.5, scalar2=1.5, op0=ALU.mult, op1=ALU.add
            )
            rr = stats.tile([P, 2], f32)
            nc.vector.tensor_tensor(out=rr, in0=r0, in1=t, op=ALU.mult)

            a_t = stats.tile([P, 2], f32)
            nc.vector.tensor_tensor(out=a_t, in0=gb_sb[:, 0:2], in1=rr, op=ALU.mult)
            b_t = stats.tile([P, 2], f32)
            nc.vector.tensor_tensor(out=b_t, in0=mean, in1=a_t, op=ALU.mult)
            nc.vector.tensor_tensor(
                out=b_t, in0=gb_sb[:, 2:4], in1=b_t, op=ALU.subtract
            )

            # ---- phase 3: normalize + relu + store (split ACT / DVE) ----
            for b in range(B_PER):
                for oc in range(2):
                    zt = z_tiles[(b, oc)]
                    if (b + oc) % 2 == 0:
                        nc.scalar.activation(
                            out=zt,
                            in_=zt,
                            func=AF.Relu,
                            scale=a_t[:, oc : oc + 1],
                            bias=b_t[:, oc : oc + 1],
                        )
                    else:
                        nc.vector.tensor_scalar(
                            out=zt,
                            in0=zt,
                            scalar1=a_t[:, oc : oc + 1],
                            scalar2=b_t[:, oc : oc + 1],
                            op0=ALU.mult,
                            op1=ALU.add,
                        )
                        nc.vector.tensor_scalar(
                            out=zt, in0=zt, scalar1=0.0, scalar2=None, op0=ALU.max
                        )
                    nc.sync.dma_start(
                        out=out_ap[b, oc * P : (oc + 1) * P, :], in_=zt
                    )

    nc.compile()
    return nc


def _prepare_aux(dw_w, mix_w, gamma, beta):
    import ml_dtypes

    # lhsT chunk for (oc, k): (mix_w[oc*128:(oc+1)*128] * dw_w[:,0,k]).T -> [C_in, 128]
    dw = np.asarray(dw_w, dtype=np.float32)  # [C_in, 1, 3]
    mw = np.asarray(mix_w, dtype=np.float32)  # [C_out, C_in]
    chunks = []
    for oc in range(2):
        for k in range(3):
            wk = mw[oc * P : (oc + 1) * P, :] * dw[None, :, 0, k]  # [128, C_in]
            chunks.append(np.ascontiguousarray(wk.T))  # [C_in, 128]
    wt = np.concatenate(chunks, axis=1).astype(ml_dtypes.bfloat16)  # [C_in, 768]
    g = np.asarray(gamma, dtype=np.float32)
    bt = np.asarray(beta, dtype=np.float32)
    gb = np.stack([g[:P], g[P:], bt[:P], bt[P:]], axis=1).astype(np.float32)
    return np.ascontiguousarray(wt), np.ascontiguousarray(gb)


def kernel(x, dw_w, dw_b, mix_w, mix_b, gamma, beta):
    import ml_dtypes

    from concourse import bass_utils

    x = np.asarray(x, dtype=np.float32)
    x_pad = np.zeros((B, C_IN, LPAD), dtype=ml_dtypes.bfloat16)
    x_pad[:, :, 1 : 1 + L] = x.astype(ml_dtypes.bfloat16)
    wt, gb = _prepare_aux(dw_w, mix_w, gamma, beta)

    if "nc" not in _CACHE:
        _CACHE["nc"] = _build_nc()
    nc = _CACHE["nc"]

    in_maps = [
        {
            "x": np.ascontiguousarray(x_pad[r * B_PER : (r + 1) * B_PER]),
            "wt": wt,
            "gb": gb,
        }
        for r in range(N_CORES)
    ]
    import os

    extra = {}
    if os.environ.get("BASS_TRACE_ALL") == "1":
        extra = {"trace_cores": list(range(N_CORES)), "stitch_traces": True}
    res = bass_utils.run_bass_kernel_spmd(
        nc, in_maps, core_ids=list(range(N_CORES)), **extra
    )
    _CACHE["last_results"] = res
    out = np.concatenate([res.results[r]["out"] for r in range(N_CORES)], axis=0)
    return out


# revision 27
# speedup vs baseline: 1.4522x; 1.4296x over previous
"""Trainium2 Bass kernel for nn_ChannelMixingConv1D.

Reference computation (B=64, C_in=128, C_out=256, L=2048, fp32):
    y = depthwise_conv1d(x, dw_w, k=3, pad=SAME) + dw_b          # [B, C_in, L]
    z = mix_w @ y + mix_b                                        # [B, C_out, L]
    out = relu(batchnorm(z) * gamma + beta)    # BN over (batch, length), biased var

Kernel strategy (8 NeuronCores, data-parallel over batch, 8 batches/core):
  * Fold the depthwise conv into the 1x1 mix:
        z[b,o,l] = sum_k sum_c (mix_w[o,c] * dw_w[c,k]) * x[b,c,l+k-1]
    i.e. 3 shifted matmuls accumulating in PSUM with host-prefolded weights.
  * The conv biases (dw_b, mix_b) shift per-channel means only, which BN
    subtracts exactly -> they drop out and are never computed.
  * Matmuls run in bf16 (x and the folded weights are converted on host):
    full PE rate + fast weight load; f32r measured ~2x slower per matmul.
  * Per (batch, out-chunk) tile: 12 matmuls -> PSUM [128, 2048];
    DVE evacuates PSUM->SBUF with a sum(z) accumulator while ACT computes
    Square with a sum(z^2) accumulator, both reading PSUM in parallel.
  * BN batch stats: tiny [128, 4] AllGather across the 8 cores (cheaper than
    AllReduce) + local reduction; a dummy warm-up collective runs under
    phase 1 to hide the ncfw wake-up latency.
  * rsqrt = ACT Sqrt + DVE reciprocal + one Newton step (ACT sqrt has a loose
    ULP budget).
  * Final out = relu(z*a + b): fused ACT activation for half the tiles, DVE
    two-op path for the other half, overlapped with the output DMA.
"""

import numpy as np

B, C_IN, C_OUT, L = 64, 128, 256, 2048
N_CORES = 8
B_PER = B // N_CORES  # 8 batches per core
EPS = 1e-5
N_TOT = float(B * L)  # BN sample count per channel
# Per-device BN stats (the sharding hint explicitly allows sync-free
# per-device stats). Saves the collective + cross-core skew absorption;
# measured end-to-end rel err ~9.5e-3 vs ~2.3e-3 with the exact all-reduce.
SYNC_FREE = True
P = 128
LPAD = L + 2  # one zero column of padding each side
N_LC = L // 512  # 4 free-dim chunks of 512

_CACHE = {}


def _build_nc():
    import concourse.bacc as bacc
    import concourse.tile as tile
    from concourse import mybir

    f32 = mybir.dt.float32
    bf16 = mybir.dt.bfloat16
    AF = mybir.ActivationFunctionType
    ALU = mybir.AluOpType

    nc = bacc.Bacc("TRN2", debug=False, num_devices=N_CORES)

    # x arrives host-padded with one zero column each side, pre-cast to bf16.
    x_d = nc.dram_tensor("x", [B_PER, C_IN, LPAD], bf16, kind="ExternalInput")
    # Pre-folded lhsT weights: wt[:, (oc*3+k)*128 : +128] = (mix_w * dw_w[:,k]).T chunk
    wt_d = nc.dram_tensor("wt", [C_IN, 6 * P], bf16, kind="ExternalInput")
    # gamma/beta split by out-chunk: cols = [g0, g1, b0, b1]
    gb_d = nc.dram_tensor("gb", [P, 4], f32, kind="ExternalInput")
    out_d = nc.dram_tensor("out", [B_PER, C_OUT, L], f32, kind="ExternalOutput")

    x_ap = x_d.ap()
    out_ap = out_d.ap()

    with tile.TileContext(nc) as tc:
        with (
            tc.tile_pool(name="consts", bufs=1) as consts,
            tc.tile_pool(name="xin", bufs=8) as xin,
            tc.tile_pool(name="zbuf", bufs=1) as zbuf,
            tc.tile_pool(name="scr", bufs=2) as scrpool,
            tc.tile_pool(name="stats", bufs=1) as stats,
            tc.tile_pool(name="psum", bufs=2, space="PSUM") as pspool,
            tc.tile_pool(name="dram", bufs=1, space="DRAM") as dram,
        ):
            # ---- constants first (tiny; the first matmul needs wt) ----
            wt_sb = consts.tile([P, 6 * P], bf16)
            nc.sync.dma_start(out=wt_sb, in_=wt_d.ap())
            gb_sb = consts.tile([P, 4], f32)
            nc.sync.dma_start(out=gb_sb, in_=gb_d.ap())

            # ---- prefetch all 8 x batches (each gets its own slot) ----
            # alternate the two HWDGE rings so two loads stream in parallel
            x_tiles = []
            for b in range(B_PER):
                xt = xin.tile([P, LPAD], bf16, tag="xt", name=f"xt{b}")
                eng = nc.sync if b % 2 == 0 else nc.scalar
                eng.dma_start(out=xt, in_=x_ap[b])
                x_tiles.append(xt)

            if not SYNC_FREE:
                # ---- warm-up collective: wakes ncfw while phase 1 runs ----
                warm_in = dram.tile([P, 1], f32)
                warm_out = dram.tile([P * N_CORES, 1], f32)
                nc.gpsimd.dma_start(out=warm_in, in_=gb_d.ap()[:, 0:1])
                nc.gpsimd.collective_compute(
                    "AllGather",
                    ALU.bypass,
                    replica_groups=[list(range(N_CORES))],
                    ins=[warm_in.opt()],
                    outs=[warm_out.opt()],
                )

            # per-(oc) accumulator columns for sum(z) / sum(z^2), one per batch
            zsum = [stats.tile([P, B_PER], f32, name=f"zsum{oc}") for oc in range(2)]
            qsum = [stats.tile([P, B_PER], f32, name=f"qsum{oc}") for oc in range(2)]

            # ---- phase 1: matmuls + evacuation + stats ----
            z_tiles = {}
            for b in range(B_PER):
                xt = x_tiles[b]
                for oc in range(2):
                    pt = pspool.tile([P, L], f32, tag="pt")
                    for lc in range(N_LC):
                        for k in range(3):
                            nc.tensor.matmul(
                                out=pt[:, lc * 512 : (lc + 1) * 512],
                                lhsT=wt_sb[
                                    :, (oc * 3 + k) * P : (oc * 3 + k + 1) * P
                                ],
                                rhs=xt[:, lc * 512 + k : lc * 512 + k + 512],
                                start=(k == 0),
                                stop=(k == 2),
                            )
                    zt = zbuf.tile([P, L], f32, tag=f"z{b}_{oc}", name=f"z{b}_{oc}")
                    z_tiles[(b, oc)] = zt
                    # DVE: copy PSUM->SBUF, accumulate sum(z)
                    nc.vector.tensor_scalar(
                        out=zt,
                        in0=pt,
                        scalar1=0.0,
                        scalar2=None,
                        op0=ALU.add,
                        op1=ALU.add,  # reduce op for accum_out
                        accum_out=zsum[oc][:, b : b + 1],
                    )
                    # ACT: z^2 into scratch, accumulate sum(z^2). Reads the
                    # SBUF copy (not PSUM) so the PSUM slot is released by
                    # the DVE evacuation alone -- keeps the PE from stalling
                    # on ACT at batch boundaries.
                    scr = scrpool.tile([P, L], f32, tag="scr")
                    nc.scalar.activation(
                        out=scr,
                        in_=zt,
                        func=AF.Square,
                        accum_out=qsum[oc][:, b : b + 1],
                    )

            # ---- phase 2: combine per-batch sums, all-gather, BN constants ----
            part = stats.tile([P, 4], f32)
            nc.vector.tensor_reduce(
                out=part[:, 0:1], in_=zsum[0], axis=mybir.AxisListType.X, op=ALU.add
            )
            nc.vector.tensor_reduce(
                out=part[:, 1:2], in_=zsum[1], axis=mybir.AxisListType.X, op=ALU.add
            )
            nc.vector.tensor_reduce(
                out=part[:, 2:3], in_=qsum[0], axis=mybir.AxisListType.X, op=ALU.add
            )
            nc.vector.tensor_reduce(
                out=part[:, 3:4], in_=qsum[1], axis=mybir.AxisListType.X, op=ALU.add
            )

            if SYNC_FREE:
                # per-device batch stats (blessed by the sharding hint):
                # no cross-core exchange at all
                tot = part
                n_stat = float(B_PER * L)
            else:
                cc_in = dram.tile([P, 4], f32)
                cc_out = dram.tile([P * N_CORES, 4], f32)
                # SWDGE so the bounce write and the collective trigger share
                # the gpsimd queue (no extra cross-engine hop)
                nc.gpsimd.dma_start(out=cc_in, in_=part)
                nc.gpsimd.collective_compute(
                    "AllGather",
                    ALU.bypass,
                    replica_groups=[list(range(N_CORES))],
                    ins=[cc_in.opt()],
                    outs=[cc_out.opt()],
                )
                # gathered layout row-major [(r p), c] -> SBUF [p, r, c]
                allp = stats.tile([P, N_CORES, 4], f32)
                nc.sync.dma_start(
                    out=allp, in_=cc_out.rearrange("(r p) c -> p r c", p=P)
                )
                tot = stats.tile([P, 4], f32)
                # reduce over cores: view as [p, c, r] and reduce innermost
                nc.vector.tensor_reduce(
                    out=tot,
                    in_=allp.transpose([0, 2, 1]),
                    axis=mybir.AxisListType.X,
                    op=ALU.add,
                )
                n_stat = N_TOT

            # mean, E[z^2] -> var -> rsqrt (Newton-refined) -> a, b
            mean = stats.tile([P, 2], f32)
            nc.vector.tensor_scalar(
                out=mean, in0=tot[:, 0:2], scalar1=1.0 / n_stat, scalar2=None,
                op0=ALU.mult,
            )
            vpe = stats.tile([P, 2], f32)
            nc.vector.tensor_scalar(
                out=vpe, in0=tot[:, 2:4], scalar1=1.0 / n_stat, scalar2=EPS,
                op0=ALU.mult, op1=ALU.add,
            )
            msq = stats.tile([P, 2], f32)
            nc.vector.tensor_tensor(out=msq, in0=mean, in1=mean, op=ALU.mult)
            nc.vector.tensor_tensor(out=vpe, in0=vpe, in1=msq, op=ALU.subtract)

            std = stats.tile([P, 2], f32)
            nc.scalar.activation(out=std, in_=vpe, func=AF.Sqrt)
            r0 = stats.tile([P, 2], f32)
            nc.vector.reciprocal(out=r0, in_=std)
            # one Newton step for rsqrt: r = r0 * (1.5 - 0.5 * v * r0^2)
            t = stats.tile([P, 2], f32)
            nc.vector.tensor_tensor(out=t, in0=vpe, in1=r0, op=ALU.mult)
            nc.vector.tensor_tensor(out=t, in0=t, in1=r0, op=ALU.mult)
            nc.vector.tensor_scalar(
                out=t, in0=t, scalar1=-0.5, scalar2=1.5, op0=ALU.mult, op1=ALU.add
            )
            rr = stats.tile([P, 2], f32)
            nc.vector.tensor_tensor(out=rr, in0=r0, in1=t, op=ALU.mult)

            a_t = stats.tile([P, 2], f32)
            nc.vector.tensor_tensor(out=a_t, in0=gb_sb[:, 0:2], in1=rr, op=ALU.mult)
            b_t = stats.tile([P, 2], f32)
            nc.vector.tensor_tensor(out=b_t, in0=mean, in1=a_t, op=ALU.mult)
            nc.vector.tensor_tensor(
                out=b_t, in0=gb_sb[:, 2:4], in1=b_t, op=ALU.subtract
            )

            # ---- phase 3: normalize + relu + store (split ACT / DVE) ----
            for b in range(B_PER):
                for oc in range(2):
                    zt = z_tiles[(b, oc)]
                    if (b + oc) % 2 == 0:
                        nc.scalar.activation(
                            out=zt,
                            in_=zt,
                            func=AF.Relu,
                            scale=a_t[:, oc : oc + 1],
                            bias=b_t[:, oc : oc + 1],
                        )
                    else:
                        nc.vector.tensor_scalar(
                            out=zt,
                            in0=zt,
                            scalar1=a_t[:, oc : oc + 1],
                            scalar2=b_t[:, oc : oc + 1],
                            op0=ALU.mult,
                            op1=ALU.add,
                        )
                        nc.vector.tensor_scalar(
                            out=zt, in0=zt, scalar1=0.0, scalar2=None, op0=ALU.max
                        )
                    nc.sync.dma_start(
                        out=out_ap[b, oc * P : (oc + 1) * P, :], in_=zt
                    )

    nc.compile()
    return nc


def _prepare_aux(dw_w, mix_w, gamma, beta):
    import ml_dtypes

    # lhsT chunk for (oc, k): (mix_w[oc*128:(oc+1)*128] * dw_w[:,0,k]).T -> [C_in, 128]
    dw = np.asarray(dw_w, dtype=np.float32)  # [C_in, 1, 3]
    mw = np.asarray(mix_w, dtype=np.float32)  # [C_out, C_in]
    chunks = []
    for oc in range(2):
        for k in range(3):
            wk = mw[oc * P : (oc + 1) * P, :] * dw[None, :, 0, k]  # [128, C_in]
            chunks.append(np.ascontiguousarray(wk.T))  # [C_in, 128]
    wt = np.concatenate(chunks, axis=1).astype(ml_dtypes.bfloat16)  # [C_in, 768]
    g = np.asarray(gamma, dtype=np.float32)
    bt = np.asarray(beta, dtype=np.float32)
    gb = np.stack([g[:P], g[P:], bt[:P], bt[P:]], axis=1).astype(np.float32)
    return np.ascontiguousarray(wt), np.ascontiguousarray(gb)


def kernel(x, dw_w, dw_b, mix_w, mix_b, gamma, beta):
    import ml_dtypes

    from concourse import bass_utils

    x = np.asarray(x, dtype=np.float32)
    x_pad = np.zeros((B, C_IN, LPAD), dtype=ml_dtypes.bfloat16)
    x_pad[:, :, 1 : 1 + L] = x.astype(ml_dtypes.bfloat16)
    wt, gb = _prepare_aux(dw_w, mix_w, gamma, beta)

    if "nc" not in _CACHE:
        _CACHE["nc"] = _build_nc()
    nc = _CACHE["nc"]

    in_maps = [
        {
            "x": np.ascontiguousarray(x_pad[r * B_PER : (r + 1) * B_PER]),
            "wt": wt,
            "gb": gb,
        }
        for r in range(N_CORES)
    ]
    import os

    extra = {}
    if os.environ.get("BASS_TRACE_ALL") == "1":
        extra = {"trace_cores": list(range(N_CORES)), "stitch_traces": True}
    res = bass_utils.run_bass_kernel_spmd(
        nc, in_maps, core_ids=list(range(N_CORES)), **extra
    )
    _CACHE["last_results"] = res
    out = np.concatenate([res.results[r]["out"] for r in range(N_CORES)], axis=0)
    return out
